# revision 11
# baseline (speedup 1.0000x reference)
"""TRN2 Bass/Tile kernel: GRU-modulated 3x3 conv (B=8, C=512, 64x64).

Sharding: data-parallel over batch across 8 NeuronCores (1 sample/core).
Each core redundantly computes the tiny GRU + affine + softmax style path
for the full batch (needs ~15us of PE) and then runs its own sample's
512->512 3x3 conv as an implicit GEMM: 9 taps x 4 ci-chunks accumulated
into PSUM over a zero-padded 66x66 input image held in SBUF.

The per-sample style scale and the (style-independent) demodulation factor
are folded into the conv weights on-device; conv matmuls run in float32r
(single-pass FP22 matmul, full PE rate at N=512).

Self-contained: hardcodes all shapes; host-side numpy does only layout
prep (transposes / slicing / gather).
"""

import numpy as np

import concourse.bacc as bacc
import concourse.mybir as mybir
from concourse import tile, masks
from concourse.bass_utils import run_bass_kernel_spmd

F32 = mybir.dt.float32
F32R = mybir.dt.float32r
AF = mybir.ActivationFunctionType
ALU = mybir.AluOpType
AX = mybir.AxisListType

B, CIN, COUT, KK, H, W = 8, 512, 512, 3, 64, 64
HID = 512
EPS = 1e-8
AFF_SCALE = float(1.0 / np.sqrt(HID))
NC4 = 4           # 512 / 128 chunks
HP, WP = H + 2, W + 2   # 66, 66
NROW = 8          # image rows per psum tile (8*64 = 512 = one PSUM bank)

_CACHE = {}
LAST_RESULTS = None


def _build():
    nc = bacc.Bacc("TRN2", target_bir_lowering=False, debug=False, num_devices=8)

    # ---- DRAM I/O ------------------------------------------------------
    xin = nc.dram_tensor("xin", [CIN, H, W], F32R, kind="ExternalInput").ap()
    wt_d = nc.dram_tensor("wt", [NC4, 128, 9, COUT], F32R, kind="ExternalInput").ap()
    wih = nc.dram_tensor("wih", [HID, 3 * HID], F32, kind="ExternalInput").ap()
    whh = nc.dram_tensor("whh", [HID, 3 * HID], F32, kind="ExternalInput").ap()
    afw = nc.dram_tensor("afw", [HID, CIN], F32, kind="ExternalInput").ap()
    bih = nc.dram_tensor("bih", [1, 3 * HID], F32, kind="ExternalInput").ap()
    bhh = nc.dram_tensor("bhh", [1, 3 * HID], F32, kind="ExternalInput").ap()
    afb = nc.dram_tensor("afb", [1, CIN], F32, kind="ExternalInput").ap()
    x8 = nc.dram_tensor("x8", [B, HID], F32, kind="ExternalInput").ap()
    h18 = nc.dram_tensor("h18", [B, HID], F32, kind="ExternalInput").ap()
    bsel = nc.dram_tensor("bsel", [B, 1], F32, kind="ExternalInput").ap()

    yout = nc.dram_tensor("yout", [COUT, H, W], F32, kind="ExternalOutput").ap()
    h2o = nc.dram_tensor("h2o", [B, HID], F32, kind="ExternalOutput").ap()
    stylo = nc.dram_tensor("stylo", [B, CIN], F32, kind="ExternalOutput").ap()

    yflat = yout.rearrange("co h w -> co (h w)")

    with tile.TileContext(nc) as tc:
        with tc.tile_pool(name="big", bufs=1) as big, \
             tc.tile_pool(name="gw", bufs=2) as gw, \
             tc.tile_pool(name="sq", bufs=2) as sqp, \
             tc.tile_pool(name="st", bufs=3) as stp:

            # ---- constants + small input DMAs (critical path first) ----
            ident = big.tile([128, 128], F32)
            masks.make_identity(nc, ident)
            ones1 = big.tile([1, B], F32)
            nc.vector.memset(ones1, 1.0)
            epsb = big.tile([128, 1], F32)
            nc.vector.memset(epsb, EPS)
            # f32r matmuls drop the start=True (first) contribution on HW:
            # clear each PSUM accumulation group with a zero-weight dummy.
            zw = big.tile([1, 512], F32R)
            nc.vector.memset(zw.bitcast(F32), 0.0)

            x8_sb = big.tile([B, HID], F32)
            nc.sync.dma_start(x8_sb, x8)
            h1_sb = big.tile([B, HID], F32)
            nc.sync.dma_start(h1_sb, h18)
            bih_sb = big.tile([1, 3 * HID], F32)
            nc.sync.dma_start(bih_sb, bih)
            bhh_sb = big.tile([1, 3 * HID], F32)
            nc.sync.dma_start(bhh_sb, bhh)
            afb_sb = big.tile([1, CIN], F32)
            nc.sync.dma_start(afb_sb, afb)
            bsel_sb = big.tile([B, 1], F32)
            nc.sync.dma_start(bsel_sb, bsel)

            # ---- big DMAs: conv weights + padded input -----------------
            wt_all = big.tile([128, NC4, 9, COUT], F32R)
            for c in range(NC4):
                nc.sync.dma_start(wt_all[:, c, :, :], wt_d[c])

            xpad = big.tile([128, NC4, HP, WP], F32R)
            nc.vector.memset(xpad[:, :, 0, :].bitcast(F32), 0.0)
            nc.vector.memset(xpad[:, :, HP - 1, :].bitcast(F32), 0.0)
            nc.gpsimd.memset(xpad[:, :, :, 0].bitcast(F32), 0.0)
            nc.gpsimd.memset(xpad[:, :, :, WP - 1].bitcast(F32), 0.0)
            xin4 = xin.rearrange("(c p) h w -> c p h w", p=128)
            for c in range(NC4):
                nc.sync.dma_start(xpad[:, c, 1:1 + H, 1:1 + W], xin4[c])

            # ---- GRU + affine + softmax(style) -------------------------
            xT = big.tile([128, NC4, B], F32)
            h1T = big.tile([128, NC4, B], F32)
            h2T = big.tile([128, NC4, B], F32)
            selc = big.tile([128, NC4], F32)

            r_sb = big.tile([B, HID], F32)
            z_sb = big.tile([B, HID], F32)
            n_sb = big.tile([B, HID], F32)
            h2_sb = big.tile([B, HID], F32)
            negmax = big.tile([B, 1], F32)
            sumexp = big.tile([B, 1], F32)
            recip = big.tile([B, 1], F32)

            with tc.tile_pool(name="pg", bufs=1, space="PSUM") as pg:
                for c in range(NC4):
                    tp = pg.tile([128, B], F32, tag="tp", bufs=2, name=f"tpx{c}")
                    nc.tensor.transpose(tp, x8_sb[0:B, c * 128:(c + 1) * 128],
                                        ident[0:B, 0:B])
                    nc.scalar.copy(xT[:, c, :], tp)
                for c in range(NC4):
                    tp = pg.tile([128, B], F32, tag="tp", bufs=2, name=f"tph{c}")
                    nc.tensor.transpose(tp, h1_sb[0:B, c * 128:(c + 1) * 128],
                                        ident[0:B, 0:B])
                    nc.scalar.copy(h1T[:, c, :], tp)

                gx = pg.tile([B, 3 * HID], F32, tag="g", bufs=2, name="gx")
                gh = pg.tile([B, 3 * HID], F32, tag="g", bufs=2, name="gh")

                for (g_ps, w_dram, lhsT, b_sb) in (
                        (gx, wih, xT, bih_sb), (gh, whh, h1T, bhh_sb)):
                    for c in range(NC4):
                        w_c = gw.tile([128, 3 * HID], F32, tag="gw", name=f"gwc{c}")
                        nc.sync.dma_start(w_c, w_dram[c * 128:(c + 1) * 128, :])
                        for jt in range(3):
                            nc.tensor.matmul(
                                g_ps[:, jt * 512:(jt + 1) * 512],
                                lhsT[:, c, :],
                                w_c[:, jt * 512:(jt + 1) * 512],
                                start=(c == 0), stop=False)
                    for jt in range(3):
                        nc.tensor.matmul(
                            g_ps[:, jt * 512:(jt + 1) * 512],
                            ones1,
                            b_sb[0:1, jt * 512:(jt + 1) * 512],
                            start=False, stop=True)

                # tensor_tensor can read at most one PSUM operand: move gx to SBUF
                gx_sb = big.tile([B, 3 * HID], F32)
                nc.scalar.copy(gx_sb, gx)
                xr, xz, xn = (gx_sb[:, i * 512:(i + 1) * 512] for i in range(3))
                hr, hz, hn = (gh[:, i * 512:(i + 1) * 512] for i in range(3))
                nc.vector.tensor_add(r_sb, xr, hr)
                nc.scalar.activation(r_sb, r_sb, AF.Sigmoid)
                nc.vector.tensor_add(z_sb, xz, hz)
                nc.scalar.activation(z_sb, z_sb, AF.Sigmoid)
                nc.vector.tensor_mul(r_sb, r_sb, hn)       # r <- r * hn
                nc.vector.tensor_add(n_sb, xn, r_sb)
                nc.scalar.activation(n_sb, n_sb, AF.Tanh)
                nc.vector.tensor_sub(h1_sb, h1_sb, n_sb)   # h1 <- h1 - n
                nc.vector.tensor_mul(h1_sb, z_sb, h1_sb)   # h1 <- z * (h1 - n)
                nc.vector.tensor_add(h2_sb, n_sb, h1_sb)   # h2 = n + z*(h1-n)
                nc.sync.dma_start(h2o, h2_sb)

                for c in range(NC4):
                    tp = pg.tile([128, B], F32, tag="tp", bufs=2, name=f"tp2{c}")
                    nc.tensor.transpose(tp, h2_sb[0:B, c * 128:(c + 1) * 128],
                                        ident[0:B, 0:B])
                    nc.scalar.activation(h2T[:, c, :], tp, AF.Copy,
                                         scale=AFF_SCALE)

                s_ps = pg.tile([B, CIN], F32, tag="tp", bufs=2, name="s_ps")
                for c in range(NC4):
                    af_c = gw.tile([128, CIN], F32, tag="gw", name=f"afc{c}")
                    nc.sync.dma_start(af_c, afw[c * 128:(c + 1) * 128, :])
                    nc.tensor.matmul(s_ps, h2T[:, c, :],
                                     af_c,
                                     start=(c == 0), stop=False)
                nc.tensor.matmul(s_ps, ones1, afb_sb,
                                 start=False, stop=True)

                # reuse r_sb (exp) and z_sb (style) — both dead after gates
                e_sb, sty_sb = r_sb, z_sb
                nc.vector.tensor_reduce(negmax, s_ps, axis=AX.X, op=ALU.max,
                                        negate=True)
                nc.scalar.activation(e_sb, s_ps, AF.Exp, bias=negmax,
                                     accum_out=sumexp)
                nc.vector.reciprocal(recip, sumexp)
                nc.vector.tensor_scalar_mul(sty_sb, e_sb, recip)
                nc.scalar.activation(sty_sb, sty_sb, AF.Sqrt)
                nc.sync.dma_start(stylo, sty_sb)

                # per-core style column: selc[:, c] = style[my_b, c*128:...]
                sel_ps = pg.tile([128, NC4], F32, tag="tp", bufs=2, name="sel_ps")
                for c in range(NC4):
                    nc.tensor.matmul(sel_ps[:, c:c + 1],
                                     sty_sb[0:B, c * 128:(c + 1) * 128],
                                     bsel_sb, start=True, stop=True)
                nc.scalar.copy(selc, sel_ps)

            # ---- demodulation + style fold into weights ----------------
            acc = big.tile([128, NC4, COUT], F32)
            for c in range(NC4):
                acc_c = acc[:, c, :]
                nc.scalar.square(acc_c, wt_all[:, c, 0, :].bitcast(F32))
                for k in range(1, 9):
                    tmp = sqp.tile([128, COUT], F32, tag="sq", name=f"sq{c}_{k}")
                    nc.scalar.square(tmp, wt_all[:, c, k, :].bitcast(F32))
                    nc.vector.tensor_add(acc_c, acc_c, tmp)
                nc.scalar.activation(acc_c, acc_c, AF.Sqrt, bias=epsb)
                nc.vector.reciprocal(acc_c, acc_c)
                for k in range(9):
                    nc.vector.scalar_tensor_tensor(
                        out=wt_all[:, c, k, :], in0=wt_all[:, c, k, :].bitcast(F32),
                        scalar=selc[:, c:c + 1], in1=acc_c,
                        op0=ALU.mult, op1=ALU.mult)

            # ---- conv: implicit GEMM -----------------------------------
            with tc.tile_pool(name="pc", bufs=8, space="PSUM") as pc:
                for m in range(NC4):            # co chunk
                    for half in range(2):       # 4 psum banks per half
                        ns = [half * 4 + i for i in range(4)]
                        ps = {n: pc.tile([128, 512], F32, tag="conv",
                                         name=f"cps{m}_{n}") for n in ns}
                        for n in ns:
                            nc.tensor.matmul(ps[n], zw[0:1, 0:128],
                                             zw[0:1, 0:512],
                                             start=True, stop=False)
                        step = 0
                        for k in range(9):
                            ky, kx = k // 3, k % 3
                            for c in range(NC4):
                                lhsT = wt_all[:, c, k,
                                              m * 128:(m + 1) * 128]
                                for n in ns:
                                    rhs = xpad[:, c,
                                               ky + n * NROW: ky + n * NROW + NROW,
                                               kx: kx + W]
                                    nc.tensor.matmul(ps[n], lhsT, rhs,
                                                     start=False,
                                                     stop=(step == 35))
                            step += 1
                        for n in ns:
                            stage = stp.tile([128, 512], F32, tag="stage",
                                             name=f"stg{m}_{n}")
                            nc.vector.tensor_copy(stage, ps[n])
                            nc.sync.dma_start(
                                yflat[m * 128:(m + 1) * 128,
                                      n * 512:(n + 1) * 512], stage)

    nc.compile()
    return nc


def _prep(h1, input, current_w_style, conv_weight, gru_w_ih, gru_w_hh,
          gru_b_ih, gru_b_hh, aff_w, aff_b):
    f = np.float32
    # conv weight -> [ci_chunk, ci_in_chunk(128), tap(9), co] contiguous
    wt = np.ascontiguousarray(
        conv_weight.astype(f).transpose(2, 3, 1, 0)      # (ky, kx, ci, co)
        .reshape(9, NC4, 128, COUT).transpose(1, 2, 0, 3))
    common = {
        "wt": wt,
        "wih": np.ascontiguousarray(gru_w_ih.astype(f).T),
        "whh": np.ascontiguousarray(gru_w_hh.astype(f).T),
        "afw": np.ascontiguousarray(aff_w.astype(f).T),
        "bih": np.ascontiguousarray(gru_b_ih.astype(f).reshape(1, -1)),
        "bhh": np.ascontiguousarray(gru_b_hh.astype(f).reshape(1, -1)),
        "afb": np.ascontiguousarray(aff_b.astype(f).reshape(1, -1)),
        "x8": np.ascontiguousarray(current_w_style.astype(f)),
        "h18": np.ascontiguousarray(h1.astype(f)),
    }
    in_maps = []
    for b in range(B):
        sel = np.zeros((B, 1), f)
        sel[b, 0] = 1.0
        m = dict(common)
        m["xin"] = np.ascontiguousarray(input[b].astype(f))
        m["bsel"] = sel
        in_maps.append(m)
    return in_maps


def kernel(h1, input, current_w_style, conv_weight, gru_w_ih, gru_w_hh,
           gru_b_ih, gru_b_hh, aff_w, aff_b):
    global LAST_RESULTS
    if "nc" not in _CACHE:
        _CACHE["nc"] = _build()
    nc = _CACHE["nc"]
    in_maps = _prep(h1, input, current_w_style, conv_weight, gru_w_ih,
                    gru_w_hh, gru_b_ih, gru_b_hh, aff_w, aff_b)
    res = run_bass_kernel_spmd(nc, in_maps, core_ids=list(range(B)))
    LAST_RESULTS = res
    out = np.stack([res.results[b]["yout"] for b in range(B)], axis=0)
    h2 = res.results[0]["h2o"]
    style = res.results[0]["stylo"]
    return h2, out, style


# revision 14
# speedup vs baseline: 1.1941x; 1.1941x over previous
"""TRN2 Bass/Tile kernel: GRU-modulated 3x3 conv (B=8, C=512, 64x64).

Sharding: data-parallel over batch across 8 NeuronCores (1 sample/core).
Each core redundantly computes the tiny GRU + affine + softmax style path
for the full batch (needs ~15us of PE) and then runs its own sample's
512->512 3x3 conv as an implicit GEMM: 9 taps x 4 ci-chunks accumulated
into PSUM over a zero-padded 66x66 input image held in SBUF.

The per-sample style scale and the (style-independent) demodulation factor
are folded into the conv weights on-device; conv matmuls run in float32r
(single-pass FP22 matmul, full PE rate at N=512).

Self-contained: hardcodes all shapes; host-side numpy does only layout
prep (transposes / slicing / gather).
"""

import numpy as np

import concourse.bacc as bacc
import concourse.mybir as mybir
from concourse import tile, masks
from concourse.bass_utils import run_bass_kernel_spmd

F32 = mybir.dt.float32
F32R = mybir.dt.float32r
BF16 = mybir.dt.bfloat16
AF = mybir.ActivationFunctionType
ALU = mybir.AluOpType
AX = mybir.AxisListType

B, CIN, COUT, KK, H, W = 8, 512, 512, 3, 64, 64
HID = 512
EPS = 1e-8
AFF_SCALE = float(1.0 / np.sqrt(HID))
NC4 = 4           # 512 / 128 chunks
HP, WP = H + 2, W + 2   # 66, 66
NROW = 8          # image rows per psum tile (8*64 = 512 = one PSUM bank)

_CACHE = {}
LAST_RESULTS = None


def _build():
    nc = bacc.Bacc("TRN2", target_bir_lowering=False, debug=False, num_devices=8)

    # ---- DRAM I/O ------------------------------------------------------
    xin = nc.dram_tensor("xin", [CIN, H, W], BF16, kind="ExternalInput").ap()
    wt_d = nc.dram_tensor("wt", [NC4, 128, 9, COUT], BF16, kind="ExternalInput").ap()
    wih = nc.dram_tensor("wih", [HID, 3 * HID], F32, kind="ExternalInput").ap()
    whh = nc.dram_tensor("whh", [HID, 3 * HID], F32, kind="ExternalInput").ap()
    afw = nc.dram_tensor("afw", [HID, CIN], F32, kind="ExternalInput").ap()
    bih = nc.dram_tensor("bih", [1, 3 * HID], F32, kind="ExternalInput").ap()
    bhh = nc.dram_tensor("bhh", [1, 3 * HID], F32, kind="ExternalInput").ap()
    afb = nc.dram_tensor("afb", [1, CIN], F32, kind="ExternalInput").ap()
    x8 = nc.dram_tensor("x8", [B, HID], F32, kind="ExternalInput").ap()
    h18 = nc.dram_tensor("h18", [B, HID], F32, kind="ExternalInput").ap()
    bsel = nc.dram_tensor("bsel", [B, 1], F32, kind="ExternalInput").ap()

    yout = nc.dram_tensor("yout", [COUT, H, W], F32, kind="ExternalOutput").ap()
    h2o = nc.dram_tensor("h2o", [B, HID], F32, kind="ExternalOutput").ap()
    stylo = nc.dram_tensor("stylo", [B, CIN], F32, kind="ExternalOutput").ap()

    yflat = yout.rearrange("co h w -> co (h w)")

    with tile.TileContext(nc) as tc:
        with tc.tile_pool(name="big", bufs=1) as big, \
             tc.tile_pool(name="gw", bufs=2) as gw, \
             tc.tile_pool(name="sq", bufs=2) as sqp, \
             tc.tile_pool(name="st", bufs=3) as stp:

            # ---- constants + small input DMAs (critical path first) ----
            ident = big.tile([128, 128], F32)
            masks.make_identity(nc, ident)
            ones1 = big.tile([1, B], F32)
            nc.vector.memset(ones1, 1.0)
            epsb = big.tile([128, 1], F32)
            nc.vector.memset(epsb, EPS)
            # f32r matmuls drop the start=True (first) contribution on HW:
            # clear each PSUM accumulation group with a zero-weight dummy.
            zw = big.tile([1, 512], BF16)
            nc.vector.memset(zw, 0.0)

            x8_sb = big.tile([B, HID], F32)
            nc.sync.dma_start(x8_sb, x8)
            h1_sb = big.tile([B, HID], F32)
            nc.sync.dma_start(h1_sb, h18)
            bih_sb = big.tile([1, 3 * HID], F32)
            nc.sync.dma_start(bih_sb, bih)
            bhh_sb = big.tile([1, 3 * HID], F32)
            nc.sync.dma_start(bhh_sb, bhh)
            afb_sb = big.tile([1, CIN], F32)
            nc.sync.dma_start(afb_sb, afb)
            bsel_sb = big.tile([B, 1], F32)
            nc.sync.dma_start(bsel_sb, bsel)

            # ---- GRU + affine + softmax(style) -------------------------
            xT = big.tile([128, NC4, B], F32)
            h1T = big.tile([128, NC4, B], F32)
            h2T = big.tile([128, NC4, B], F32)
            selc = big.tile([128, NC4], F32)

            r_sb = big.tile([B, HID], F32)
            z_sb = big.tile([B, HID], F32)
            n_sb = big.tile([B, HID], F32)
            h2_sb = big.tile([B, HID], F32)
            negmax = big.tile([B, 1], F32)
            sumexp = big.tile([B, 1], F32)
            recip = big.tile([B, 1], F32)

            with tc.tile_pool(name="pg", bufs=1, space="PSUM") as pg:
                for c in range(NC4):
                    tp = pg.tile([128, B], F32, tag="tp", bufs=2, name=f"tpx{c}")
                    nc.tensor.transpose(tp, x8_sb[0:B, c * 128:(c + 1) * 128],
                                        ident[0:B, 0:B])
                    nc.scalar.copy(xT[:, c, :], tp)
                for c in range(NC4):
                    tp = pg.tile([128, B], F32, tag="tp", bufs=2, name=f"tph{c}")
                    nc.tensor.transpose(tp, h1_sb[0:B, c * 128:(c + 1) * 128],
                                        ident[0:B, 0:B])
                    nc.scalar.copy(h1T[:, c, :], tp)

                gx = pg.tile([B, 3 * HID], F32, tag="g", bufs=2, name="gx")
                gh = pg.tile([B, 3 * HID], F32, tag="g", bufs=2, name="gh")

                for (g_ps, w_dram, lhsT, b_sb) in (
                        (gx, wih, xT, bih_sb), (gh, whh, h1T, bhh_sb)):
                    for c in range(NC4):
                        w_c = gw.tile([128, 3 * HID], F32, tag="gw", name=f"gwc{c}")
                        nc.sync.dma_start(w_c, w_dram[c * 128:(c + 1) * 128, :])
                        for jt in range(3):
                            nc.tensor.matmul(
                                g_ps[:, jt * 512:(jt + 1) * 512],
                                lhsT[:, c, :],
                                w_c[:, jt * 512:(jt + 1) * 512],
                                start=(c == 0), stop=False)
                    for jt in range(3):
                        nc.tensor.matmul(
                            g_ps[:, jt * 512:(jt + 1) * 512],
                            ones1,
                            b_sb[0:1, jt * 512:(jt + 1) * 512],
                            start=False, stop=True)

                # tensor_tensor can read at most one PSUM operand: move gx to SBUF
                gx_sb = big.tile([B, 3 * HID], F32)
                nc.scalar.copy(gx_sb, gx)
                xr, xz, xn = (gx_sb[:, i * 512:(i + 1) * 512] for i in range(3))
                hr, hz, hn = (gh[:, i * 512:(i + 1) * 512] for i in range(3))
                nc.vector.tensor_add(r_sb, xr, hr)
                nc.scalar.activation(r_sb, r_sb, AF.Sigmoid)
                nc.vector.tensor_add(z_sb, xz, hz)
                nc.scalar.activation(z_sb, z_sb, AF.Sigmoid)
                nc.vector.tensor_mul(r_sb, r_sb, hn)       # r <- r * hn
                nc.vector.tensor_add(n_sb, xn, r_sb)
                nc.scalar.activation(n_sb, n_sb, AF.Tanh)
                nc.vector.tensor_sub(h1_sb, h1_sb, n_sb)   # h1 <- h1 - n
                nc.vector.tensor_mul(h1_sb, z_sb, h1_sb)   # h1 <- z * (h1 - n)
                nc.vector.tensor_add(h2_sb, n_sb, h1_sb)   # h2 = n + z*(h1-n)
                nc.sync.dma_start(h2o, h2_sb)

                for c in range(NC4):
                    tp = pg.tile([128, B], F32, tag="tp", bufs=2, name=f"tp2{c}")
                    nc.tensor.transpose(tp, h2_sb[0:B, c * 128:(c + 1) * 128],
                                        ident[0:B, 0:B])
                    nc.scalar.activation(h2T[:, c, :], tp, AF.Copy,
                                         scale=AFF_SCALE)

                s_ps = pg.tile([B, CIN], F32, tag="tp", bufs=2, name="s_ps")
                for c in range(NC4):
                    af_c = gw.tile([128, CIN], F32, tag="gw", name=f"afc{c}")
                    nc.sync.dma_start(af_c, afw[c * 128:(c + 1) * 128, :])
                    nc.tensor.matmul(s_ps, h2T[:, c, :],
                                     af_c,
                                     start=(c == 0), stop=False)
                nc.tensor.matmul(s_ps, ones1, afb_sb,
                                 start=False, stop=True)

                # reuse r_sb (exp) and z_sb (style) — both dead after gates
                e_sb, sty_sb = r_sb, z_sb
                nc.vector.tensor_reduce(negmax, s_ps, axis=AX.X, op=ALU.max,
                                        negate=True)
                nc.scalar.activation(e_sb, s_ps, AF.Exp, bias=negmax,
                                     accum_out=sumexp)
                nc.vector.reciprocal(recip, sumexp)
                nc.vector.tensor_scalar_mul(sty_sb, e_sb, recip)
                nc.scalar.activation(sty_sb, sty_sb, AF.Sqrt)
                nc.sync.dma_start(stylo, sty_sb)

                # per-core style column: selc[:, c] = style[my_b, c*128:...]
                sel_ps = pg.tile([128, NC4], F32, tag="tp", bufs=2, name="sel_ps")
                for c in range(NC4):
                    nc.tensor.matmul(sel_ps[:, c:c + 1],
                                     sty_sb[0:B, c * 128:(c + 1) * 128],
                                     bsel_sb, start=True, stop=True)
                nc.scalar.copy(selc, sel_ps)

            # ---- big DMAs: conv weights + padded input, interleaved by
            # chunk so chunk 0 of both lands first (emitted after the GRU
            # weight DMAs, which gate the style critical path) ------------
            wt_all = big.tile([128, NC4, 9, COUT], BF16)
            xpad = big.tile([128, NC4, HP, WP], BF16)
            nc.vector.memset(xpad[:, :, 0, :], 0.0)
            nc.vector.memset(xpad[:, :, HP - 1, :], 0.0)
            nc.gpsimd.memset(xpad[:, :, :, 0], 0.0)
            nc.gpsimd.memset(xpad[:, :, :, WP - 1], 0.0)
            xin4 = xin.rearrange("(c p) h w -> c p h w", p=128)
            for c in range(NC4):
                nc.sync.dma_start(wt_all[:, c, :, :], wt_d[c])
                nc.sync.dma_start(xpad[:, c, 1:1 + H, 1:1 + W], xin4[c])

            # ---- demodulation + style fold into weights ----------------
            acc = big.tile([128, NC4, COUT], F32)
            for c in range(NC4):
                acc_c = acc[:, c, :]
                nc.scalar.square(acc_c, wt_all[:, c, 0, :])
                for k in range(1, 9):
                    tmp = sqp.tile([128, COUT], F32, tag="sq", name=f"sq{c}_{k}")
                    nc.scalar.square(tmp, wt_all[:, c, k, :])
                    nc.vector.tensor_add(acc_c, acc_c, tmp)
                nc.scalar.activation(acc_c, acc_c, AF.Sqrt, bias=epsb)
                nc.vector.reciprocal(acc_c, acc_c)
                for k in range(9):
                    nc.vector.scalar_tensor_tensor(
                        out=wt_all[:, c, k, :], in0=wt_all[:, c, k, :],
                        scalar=selc[:, c:c + 1], in1=acc_c,
                        op0=ALU.mult, op1=ALU.mult)

            # ---- conv: implicit GEMM -----------------------------------
            with tc.tile_pool(name="pc", bufs=8, space="PSUM") as pc:
                for m in range(NC4):            # co chunk
                    for half in range(2):       # 4 psum banks per half
                        ns = [half * 4 + i for i in range(4)]
                        ps = {n: pc.tile([128, 512], F32, tag="conv",
                                         name=f"cps{m}_{n}") for n in ns}
                        for n in ns:
                            nc.tensor.matmul(ps[n], zw[0:1, 0:128],
                                             zw[0:1, 0:512],
                                             start=True, stop=False)
                        step = 0
                        for c in range(NC4):
                            for k in range(9):
                                ky, kx = k // 3, k % 3
                                lhsT = wt_all[:, c, k,
                                              m * 128:(m + 1) * 128]
                                for n in ns:
                                    rhs = xpad[:, c,
                                               ky + n * NROW: ky + n * NROW + NROW,
                                               kx: kx + W]
                                    nc.tensor.matmul(ps[n], lhsT, rhs,
                                                     start=False,
                                                     stop=(step == 35))
                                step += 1
                        for n in ns:
                            stage = stp.tile([128, 512], F32, tag="stage",
                                             name=f"stg{m}_{n}")
                            nc.vector.tensor_copy(stage, ps[n])
                            nc.sync.dma_start(
                                yflat[m * 128:(m + 1) * 128,
                                      n * 512:(n + 1) * 512], stage)

    nc.compile()
    return nc


def _prep(h1, input, current_w_style, conv_weight, gru_w_ih, gru_w_hh,
          gru_b_ih, gru_b_hh, aff_w, aff_b):
    import ml_dtypes
    f = np.float32
    # conv weight -> [ci_chunk, ci_in_chunk(128), tap(9), co] contiguous
    wt = np.ascontiguousarray(
        conv_weight.astype(f).transpose(2, 3, 1, 0)      # (ky, kx, ci, co)
        .reshape(9, NC4, 128, COUT).transpose(1, 2, 0, 3)
        .astype(ml_dtypes.bfloat16))
    common = {
        "wt": wt,
        "wih": np.ascontiguousarray(gru_w_ih.astype(f).T),
        "whh": np.ascontiguousarray(gru_w_hh.astype(f).T),
        "afw": np.ascontiguousarray(aff_w.astype(f).T),
        "bih": np.ascontiguousarray(gru_b_ih.astype(f).reshape(1, -1)),
        "bhh": np.ascontiguousarray(gru_b_hh.astype(f).reshape(1, -1)),
        "afb": np.ascontiguousarray(aff_b.astype(f).reshape(1, -1)),
        "x8": np.ascontiguousarray(current_w_style.astype(f)),
        "h18": np.ascontiguousarray(h1.astype(f)),
    }
    in_maps = []
    for b in range(B):
        sel = np.zeros((B, 1), f)
        sel[b, 0] = 1.0
        m = dict(common)
        m["xin"] = np.ascontiguousarray(
            input[b].astype(f).astype(ml_dtypes.bfloat16))
        m["bsel"] = sel
        in_maps.append(m)
    return in_maps


def kernel(h1, input, current_w_style, conv_weight, gru_w_ih, gru_w_hh,
           gru_b_ih, gru_b_hh, aff_w, aff_b):
    global LAST_RESULTS
    if "nc" not in _CACHE:
        _CACHE["nc"] = _build()
    nc = _CACHE["nc"]
    in_maps = _prep(h1, input, current_w_style, conv_weight, gru_w_ih,
                    gru_w_hh, gru_b_ih, gru_b_hh, aff_w, aff_b)
    res = run_bass_kernel_spmd(nc, in_maps, core_ids=list(range(B)))
    LAST_RESULTS = res
    out = np.stack([res.results[b]["yout"] for b in range(B)], axis=0)
    h2 = res.results[0]["h2o"]
    style = res.results[0]["stylo"]
    return h2, out, style


# revision 15
# speedup vs baseline: 1.2841x; 1.0754x over previous
"""TRN2 Bass/Tile kernel: GRU-modulated 3x3 conv (B=8, C=512, 64x64).

Sharding: data-parallel over batch across 8 NeuronCores (1 sample/core).
Each core redundantly computes the tiny GRU + affine + softmax style path
for the full batch (needs ~15us of PE) and then runs its own sample's
512->512 3x3 conv as an implicit GEMM: 9 taps x 4 ci-chunks accumulated
into PSUM over a zero-padded 66x66 input image held in SBUF.

The per-sample style scale and the (style-independent) demodulation factor
are folded into the conv weights on-device; conv matmuls run in float32r
(single-pass FP22 matmul, full PE rate at N=512).

Self-contained: hardcodes all shapes; host-side numpy does only layout
prep (transposes / slicing / gather).
"""

import numpy as np

import concourse.bacc as bacc
import concourse.mybir as mybir
from concourse import tile, masks
from concourse.bass_utils import run_bass_kernel_spmd

F32 = mybir.dt.float32
F32R = mybir.dt.float32r
BF16 = mybir.dt.bfloat16
AF = mybir.ActivationFunctionType
ALU = mybir.AluOpType
AX = mybir.AxisListType

B, CIN, COUT, KK, H, W = 8, 512, 512, 3, 64, 64
HID = 512
EPS = 1e-8
AFF_SCALE = float(1.0 / np.sqrt(HID))
NC4 = 4           # 512 / 128 chunks
HP, WP = H + 2, W + 2   # 66, 66
NROW = 8          # image rows per psum tile (8*64 = 512 = one PSUM bank)

_CACHE = {}
LAST_RESULTS = None


def _build():
    nc = bacc.Bacc("TRN2", target_bir_lowering=False, debug=False, num_devices=8)

    # ---- DRAM I/O ------------------------------------------------------
    xin = nc.dram_tensor("xin", [CIN, H, W], BF16, kind="ExternalInput").ap()
    wt_d = nc.dram_tensor("wt", [NC4, 128, 9, COUT], BF16, kind="ExternalInput").ap()
    wih = nc.dram_tensor("wih", [HID, 3 * HID], F32, kind="ExternalInput").ap()
    whh = nc.dram_tensor("whh", [HID, 3 * HID], F32, kind="ExternalInput").ap()
    afw = nc.dram_tensor("afw", [HID, CIN], F32, kind="ExternalInput").ap()
    bih = nc.dram_tensor("bih", [1, 3 * HID], F32, kind="ExternalInput").ap()
    bhh = nc.dram_tensor("bhh", [1, 3 * HID], F32, kind="ExternalInput").ap()
    afb = nc.dram_tensor("afb", [1, CIN], F32, kind="ExternalInput").ap()
    x8 = nc.dram_tensor("x8", [B, HID], F32, kind="ExternalInput").ap()
    h18 = nc.dram_tensor("h18", [B, HID], F32, kind="ExternalInput").ap()
    bsel = nc.dram_tensor("bsel", [B, 1], F32, kind="ExternalInput").ap()

    yout = nc.dram_tensor("yout", [COUT, H, W], F32, kind="ExternalOutput").ap()
    h2o = nc.dram_tensor("h2o", [B, HID], F32, kind="ExternalOutput").ap()
    stylo = nc.dram_tensor("stylo", [B, CIN], F32, kind="ExternalOutput").ap()

    yflat = yout.rearrange("co h w -> co (h w)")

    with tile.TileContext(nc) as tc:
        with tc.tile_pool(name="big", bufs=1) as big, \
             tc.tile_pool(name="gw", bufs=5) as gw, \
             tc.tile_pool(name="sq", bufs=2) as sqp, \
             tc.tile_pool(name="st", bufs=3) as stp:

            # ---- constants + small input DMAs (critical path first) ----
            ident = big.tile([128, 128], F32)
            masks.make_identity(nc, ident)
            ones1 = big.tile([1, B], F32)
            nc.vector.memset(ones1, 1.0)
            epsb = big.tile([128, 1], F32)
            nc.vector.memset(epsb, EPS)
            # f32r matmuls drop the start=True (first) contribution on HW:
            # clear each PSUM accumulation group with a zero-weight dummy.
            zw = big.tile([1, 512], BF16)
            nc.vector.memset(zw, 0.0)

            x8_sb = big.tile([B, HID], F32)
            nc.sync.dma_start(x8_sb, x8)
            h1_sb = big.tile([B, HID], F32)
            nc.sync.dma_start(h1_sb, h18)
            bih_sb = big.tile([1, 3 * HID], F32)
            nc.sync.dma_start(bih_sb, bih)
            bhh_sb = big.tile([1, 3 * HID], F32)
            nc.sync.dma_start(bhh_sb, bhh)
            afb_sb = big.tile([1, CIN], F32)
            nc.sync.dma_start(afb_sb, afb)
            bsel_sb = big.tile([B, 1], F32)
            nc.sync.dma_start(bsel_sb, bsel)

            # ---- GRU + affine + softmax(style) -------------------------
            xT = big.tile([128, NC4, B], F32)
            h1T = big.tile([128, NC4, B], F32)
            h2T = big.tile([128, NC4, B], F32)
            selc = big.tile([128, NC4], F32)

            r_sb = big.tile([B, HID], F32)
            z_sb = big.tile([B, HID], F32)
            n_sb = big.tile([B, HID], F32)
            h2_sb = big.tile([B, HID], F32)
            negmax = big.tile([B, 1], F32)
            sumexp = big.tile([B, 1], F32)
            recip = big.tile([B, 1], F32)

            with tc.tile_pool(name="pg", bufs=1, space="PSUM") as pg:
                for c in range(NC4):
                    tp = pg.tile([128, B], F32, tag="tp", bufs=2, name=f"tpx{c}")
                    nc.tensor.transpose(tp, x8_sb[0:B, c * 128:(c + 1) * 128],
                                        ident[0:B, 0:B])
                    nc.scalar.copy(xT[:, c, :], tp)
                for c in range(NC4):
                    tp = pg.tile([128, B], F32, tag="tp", bufs=2, name=f"tph{c}")
                    nc.tensor.transpose(tp, h1_sb[0:B, c * 128:(c + 1) * 128],
                                        ident[0:B, 0:B])
                    nc.scalar.copy(h1T[:, c, :], tp)

                gx = pg.tile([B, 3 * HID], F32, tag="g", bufs=2, name="gx")
                gh = pg.tile([B, 3 * HID], F32, tag="g", bufs=2, name="gh")

                for (g_ps, w_dram, lhsT, b_sb) in (
                        (gx, wih, xT, bih_sb), (gh, whh, h1T, bhh_sb)):
                    for c in range(NC4):
                        w_c = gw.tile([128, 3 * HID], F32, tag="gw", name=f"gwc{c}")
                        nc.sync.dma_start(w_c, w_dram[c * 128:(c + 1) * 128, :])
                        for jt in range(3):
                            nc.tensor.matmul(
                                g_ps[:, jt * 512:(jt + 1) * 512],
                                lhsT[:, c, :],
                                w_c[:, jt * 512:(jt + 1) * 512],
                                start=(c == 0), stop=False)
                    for jt in range(3):
                        nc.tensor.matmul(
                            g_ps[:, jt * 512:(jt + 1) * 512],
                            ones1,
                            b_sb[0:1, jt * 512:(jt + 1) * 512],
                            start=False, stop=True)

                # tensor_tensor can read at most one PSUM operand: move gx to SBUF
                gx_sb = big.tile([B, 3 * HID], F32)
                nc.scalar.copy(gx_sb, gx)
                xr, xz, xn = (gx_sb[:, i * 512:(i + 1) * 512] for i in range(3))
                hr, hz, hn = (gh[:, i * 512:(i + 1) * 512] for i in range(3))
                nc.vector.tensor_add(r_sb, xr, hr)
                nc.scalar.activation(r_sb, r_sb, AF.Sigmoid)
                nc.vector.tensor_add(z_sb, xz, hz)
                nc.scalar.activation(z_sb, z_sb, AF.Sigmoid)
                nc.vector.tensor_mul(r_sb, r_sb, hn)       # r <- r * hn
                nc.vector.tensor_add(n_sb, xn, r_sb)
                nc.scalar.activation(n_sb, n_sb, AF.Tanh)
                nc.vector.tensor_sub(h1_sb, h1_sb, n_sb)   # h1 <- h1 - n
                nc.vector.tensor_mul(h1_sb, z_sb, h1_sb)   # h1 <- z * (h1 - n)
                nc.vector.tensor_add(h2_sb, n_sb, h1_sb)   # h2 = n + z*(h1-n)
                nc.sync.dma_start(h2o, h2_sb)

                for c in range(NC4):
                    tp = pg.tile([128, B], F32, tag="tp", bufs=2, name=f"tp2{c}")
                    nc.tensor.transpose(tp, h2_sb[0:B, c * 128:(c + 1) * 128],
                                        ident[0:B, 0:B])
                    nc.scalar.activation(h2T[:, c, :], tp, AF.Copy,
                                         scale=AFF_SCALE)

                s_ps = pg.tile([B, CIN], F32, tag="tp", bufs=2, name="s_ps")
                for c in range(NC4):
                    af_c = gw.tile([128, CIN], F32, tag="gw", name=f"afc{c}")
                    nc.sync.dma_start(af_c, afw[c * 128:(c + 1) * 128, :])
                    nc.tensor.matmul(s_ps, h2T[:, c, :],
                                     af_c,
                                     start=(c == 0), stop=False)
                nc.tensor.matmul(s_ps, ones1, afb_sb,
                                 start=False, stop=True)

                # reuse r_sb (exp) and z_sb (style) — both dead after gates
                e_sb, sty_sb = r_sb, z_sb
                nc.vector.tensor_reduce(negmax, s_ps, axis=AX.X, op=ALU.max,
                                        negate=True)
                nc.scalar.activation(e_sb, s_ps, AF.Exp, bias=negmax,
                                     accum_out=sumexp)
                nc.vector.reciprocal(recip, sumexp)
                nc.vector.tensor_scalar_mul(sty_sb, e_sb, recip)
                nc.scalar.activation(sty_sb, sty_sb, AF.Sqrt)
                nc.sync.dma_start(stylo, sty_sb)

                # per-core style column: selc[:, c] = style[my_b, c*128:...]
                sel_ps = pg.tile([128, NC4], F32, tag="tp", bufs=2, name="sel_ps")
                for c in range(NC4):
                    nc.tensor.matmul(sel_ps[:, c:c + 1],
                                     sty_sb[0:B, c * 128:(c + 1) * 128],
                                     bsel_sb, start=True, stop=True)
                nc.scalar.copy(selc, sel_ps)

            # ---- big DMAs: conv weights + padded input, interleaved by
            # chunk so chunk 0 of both lands first (emitted after the GRU
            # weight DMAs, which gate the style critical path) ------------
            wt_all = big.tile([128, NC4, 9, COUT], BF16)
            xpad = big.tile([128, NC4, HP, WP], BF16)
            nc.vector.memset(xpad[:, :, 0, :], 0.0)
            nc.vector.memset(xpad[:, :, HP - 1, :], 0.0)
            nc.vector.memset(xpad[:, :, :, 0], 0.0)
            nc.vector.memset(xpad[:, :, :, WP - 1], 0.0)
            xin4 = xin.rearrange("(c p) h w -> c p h w", p=128)
            for c in range(NC4):
                nc.sync.dma_start(wt_all[:, c, :, :], wt_d[c])
                nc.sync.dma_start(xpad[:, c, 1:1 + H, 1:1 + W], xin4[c])

            # ---- demodulation + style fold into weights ----------------
            acc = big.tile([128, NC4, COUT], F32)
            for c in range(NC4):
                acc_c = acc[:, c, :]
                nc.scalar.square(acc_c, wt_all[:, c, 0, :])
                for k in range(1, 9):
                    tmp = sqp.tile([128, COUT], F32, tag="sq", name=f"sq{c}_{k}")
                    nc.scalar.square(tmp, wt_all[:, c, k, :])
                    nc.vector.tensor_add(acc_c, acc_c, tmp)
                nc.scalar.activation(acc_c, acc_c, AF.Sqrt, bias=epsb)
                nc.vector.reciprocal(acc_c, acc_c)
                for k in range(9):
                    nc.vector.scalar_tensor_tensor(
                        out=wt_all[:, c, k, :], in0=wt_all[:, c, k, :],
                        scalar=selc[:, c:c + 1], in1=acc_c,
                        op0=ALU.mult, op1=ALU.mult)

            # ---- conv: implicit GEMM -----------------------------------
            with tc.tile_pool(name="pc", bufs=8, space="PSUM") as pc:
                for m in range(NC4):            # co chunk
                    for half in range(2):       # 4 psum banks per half
                        ns = [half * 4 + i for i in range(4)]
                        ps = {n: pc.tile([128, 512], F32, tag="conv",
                                         name=f"cps{m}_{n}") for n in ns}
                        for n in ns:
                            nc.tensor.matmul(ps[n], zw[0:1, 0:128],
                                             zw[0:1, 0:512],
                                             start=True, stop=False)
                        step = 0
                        for c in range(NC4):
                            for k in range(9):
                                ky, kx = k // 3, k % 3
                                lhsT = wt_all[:, c, k,
                                              m * 128:(m + 1) * 128]
                                for n in ns:
                                    rhs = xpad[:, c,
                                               ky + n * NROW: ky + n * NROW + NROW,
                                               kx: kx + W]
                                    nc.tensor.matmul(ps[n], lhsT, rhs,
                                                     start=False,
                                                     stop=(step == 35))
                                step += 1
                        for n in ns:
                            stage = stp.tile([128, 512], F32, tag="stage",
                                             name=f"stg{m}_{n}")
                            nc.vector.tensor_copy(stage, ps[n])
                            nc.sync.dma_start(
                                yflat[m * 128:(m + 1) * 128,
                                      n * 512:(n + 1) * 512], stage)

    nc.compile()
    return nc


def _prep(h1, input, current_w_style, conv_weight, gru_w_ih, gru_w_hh,
          gru_b_ih, gru_b_hh, aff_w, aff_b):
    import ml_dtypes
    f = np.float32
    # conv weight -> [ci_chunk, ci_in_chunk(128), tap(9), co] contiguous
    wt = np.ascontiguousarray(
        conv_weight.astype(f).transpose(2, 3, 1, 0)      # (ky, kx, ci, co)
        .reshape(9, NC4, 128, COUT).transpose(1, 2, 0, 3)
        .astype(ml_dtypes.bfloat16))
    common = {
        "wt": wt,
        "wih": np.ascontiguousarray(gru_w_ih.astype(f).T),
        "whh": np.ascontiguousarray(gru_w_hh.astype(f).T),
        "afw": np.ascontiguousarray(aff_w.astype(f).T),
        "bih": np.ascontiguousarray(gru_b_ih.astype(f).reshape(1, -1)),
        "bhh": np.ascontiguousarray(gru_b_hh.astype(f).reshape(1, -1)),
        "afb": np.ascontiguousarray(aff_b.astype(f).reshape(1, -1)),
        "x8": np.ascontiguousarray(current_w_style.astype(f)),
        "h18": np.ascontiguousarray(h1.astype(f)),
    }
    in_maps = []
    for b in range(B):
        sel = np.zeros((B, 1), f)
        sel[b, 0] = 1.0
        m = dict(common)
        m["xin"] = np.ascontiguousarray(
            input[b].astype(f).astype(ml_dtypes.bfloat16))
        m["bsel"] = sel
        in_maps.append(m)
    return in_maps


def kernel(h1, input, current_w_style, conv_weight, gru_w_ih, gru_w_hh,
           gru_b_ih, gru_b_hh, aff_w, aff_b):
    global LAST_RESULTS
    if "nc" not in _CACHE:
        _CACHE["nc"] = _build()
    nc = _CACHE["nc"]
    in_maps = _prep(h1, input, current_w_style, conv_weight, gru_w_ih,
                    gru_w_hh, gru_b_ih, gru_b_hh, aff_w, aff_b)
    res = run_bass_kernel_spmd(nc, in_maps, core_ids=list(range(B)))
    LAST_RESULTS = res
    out = np.stack([res.results[b]["yout"] for b in range(B)], axis=0)
    h2 = res.results[0]["h2o"]
    style = res.results[0]["stylo"]
    return h2, out, style


# revision 16
# speedup vs baseline: 1.3001x; 1.0125x over previous
"""TRN2 Bass/Tile kernel: GRU-modulated 3x3 conv (B=8, C=512, 64x64).

Sharding: data-parallel over batch across 8 NeuronCores (1 sample/core).
Each core redundantly computes the tiny GRU + affine + softmax style path
for the full batch (needs ~15us of PE) and then runs its own sample's
512->512 3x3 conv as an implicit GEMM: 9 taps x 4 ci-chunks accumulated
into PSUM over a zero-padded 66x66 input image held in SBUF.

The per-sample style scale and the (style-independent) demodulation factor
are folded into the conv weights on-device; conv matmuls run in float32r
(single-pass FP22 matmul, full PE rate at N=512).

Self-contained: hardcodes all shapes; host-side numpy does only layout
prep (transposes / slicing / gather).
"""

import numpy as np

import concourse.bacc as bacc
import concourse.mybir as mybir
from concourse import tile, masks
from concourse.bass_utils import run_bass_kernel_spmd

F32 = mybir.dt.float32
F32R = mybir.dt.float32r
BF16 = mybir.dt.bfloat16
F16 = mybir.dt.float16
AF = mybir.ActivationFunctionType
ALU = mybir.AluOpType
AX = mybir.AxisListType

B, CIN, COUT, KK, H, W = 8, 512, 512, 3, 64, 64
HID = 512
EPS = 1e-8
AFF_SCALE = float(1.0 / np.sqrt(HID))
NC4 = 4           # 512 / 128 chunks
HP, WP = H + 2, W + 2   # 66, 66
NROW = 8          # image rows per psum tile (8*64 = 512 = one PSUM bank)

_CACHE = {}
LAST_RESULTS = None


def _build():
    nc = bacc.Bacc("TRN2", target_bir_lowering=False, debug=False, num_devices=8)

    # ---- DRAM I/O ------------------------------------------------------
    xin = nc.dram_tensor("xin", [CIN, H, W], BF16, kind="ExternalInput").ap()
    wt_d = nc.dram_tensor("wt", [NC4, 128, 9, COUT], BF16, kind="ExternalInput").ap()
    wih = nc.dram_tensor("wih", [HID, 3 * HID], F16, kind="ExternalInput").ap()
    whh = nc.dram_tensor("whh", [HID, 3 * HID], F16, kind="ExternalInput").ap()
    afw = nc.dram_tensor("afw", [HID, CIN], F32, kind="ExternalInput").ap()
    bih = nc.dram_tensor("bih", [1, 3 * HID], F16, kind="ExternalInput").ap()
    bhh = nc.dram_tensor("bhh", [1, 3 * HID], F16, kind="ExternalInput").ap()
    afb = nc.dram_tensor("afb", [1, CIN], F32, kind="ExternalInput").ap()
    x8 = nc.dram_tensor("x8", [B, HID], F32, kind="ExternalInput").ap()
    h18 = nc.dram_tensor("h18", [B, HID], F32, kind="ExternalInput").ap()
    bsel = nc.dram_tensor("bsel", [B, 1], F32, kind="ExternalInput").ap()

    yout = nc.dram_tensor("yout", [COUT, H, W], F32, kind="ExternalOutput").ap()
    h2o = nc.dram_tensor("h2o", [B, HID], F32, kind="ExternalOutput").ap()
    stylo = nc.dram_tensor("stylo", [B, CIN], F32, kind="ExternalOutput").ap()

    yflat = yout.rearrange("co h w -> co (h w)")

    with tile.TileContext(nc) as tc:
        with tc.tile_pool(name="big", bufs=1) as big, \
             tc.tile_pool(name="gw", bufs=5) as gw, \
             tc.tile_pool(name="sq", bufs=2) as sqp, \
             tc.tile_pool(name="st", bufs=3) as stp:

            # ---- constants + small input DMAs (critical path first) ----
            ident = big.tile([128, 128], F32)
            masks.make_identity(nc, ident)
            ones1 = big.tile([1, B], F32)
            nc.vector.memset(ones1, 1.0)
            ones16 = big.tile([1, B], F16)
            nc.vector.memset(ones16, 1.0)
            epsb = big.tile([128, 1], F32)
            nc.vector.memset(epsb, EPS)
            # f32r matmuls drop the start=True (first) contribution on HW:
            # clear each PSUM accumulation group with a zero-weight dummy.
            zw = big.tile([1, 512], BF16)
            nc.vector.memset(zw, 0.0)

            x8_sb = big.tile([B, HID], F32)
            nc.sync.dma_start(x8_sb, x8)
            h1_sb = big.tile([B, HID], F32)
            nc.sync.dma_start(h1_sb, h18)
            bih_sb = big.tile([1, 3 * HID], F16)
            nc.sync.dma_start(bih_sb, bih)
            bhh_sb = big.tile([1, 3 * HID], F16)
            nc.sync.dma_start(bhh_sb, bhh)
            afb_sb = big.tile([1, CIN], F32)
            nc.sync.dma_start(afb_sb, afb)
            bsel_sb = big.tile([B, 1], F32)
            nc.sync.dma_start(bsel_sb, bsel)

            # ---- GRU + affine + softmax(style) -------------------------
            xT = big.tile([128, NC4, B], F16)
            h1T = big.tile([128, NC4, B], F16)
            h2T = big.tile([128, NC4, B], F32)
            selc = big.tile([128, NC4], F32)

            r_sb = big.tile([B, HID], F32)
            z_sb = big.tile([B, HID], F32)
            n_sb = big.tile([B, HID], F32)
            h2_sb = big.tile([B, HID], F32)
            negmax = big.tile([B, 1], F32)
            sumexp = big.tile([B, 1], F32)
            recip = big.tile([B, 1], F32)

            with tc.tile_pool(name="pg", bufs=1, space="PSUM") as pg:
                for c in range(NC4):
                    tp = pg.tile([128, B], F32, tag="tp", bufs=2, name=f"tpx{c}")
                    nc.tensor.transpose(tp, x8_sb[0:B, c * 128:(c + 1) * 128],
                                        ident[0:B, 0:B])
                    nc.scalar.copy(xT[:, c, :], tp)
                for c in range(NC4):
                    tp = pg.tile([128, B], F32, tag="tp", bufs=2, name=f"tph{c}")
                    nc.tensor.transpose(tp, h1_sb[0:B, c * 128:(c + 1) * 128],
                                        ident[0:B, 0:B])
                    nc.scalar.copy(h1T[:, c, :], tp)

                gx = pg.tile([B, 3 * HID], F32, tag="g", bufs=2, name="gx")
                gh = pg.tile([B, 3 * HID], F32, tag="g", bufs=2, name="gh")

                for (g_ps, w_dram, lhsT, b_sb) in (
                        (gx, wih, xT, bih_sb), (gh, whh, h1T, bhh_sb)):
                    for c in range(NC4):
                        w_c = gw.tile([128, 3 * HID], F16, tag="gw", name=f"gwc{c}")
                        nc.sync.dma_start(w_c, w_dram[c * 128:(c + 1) * 128, :])
                        for jt in range(3):
                            nc.tensor.matmul(
                                g_ps[:, jt * 512:(jt + 1) * 512],
                                lhsT[:, c, :],
                                w_c[:, jt * 512:(jt + 1) * 512],
                                start=(c == 0), stop=False)
                    for jt in range(3):
                        nc.tensor.matmul(
                            g_ps[:, jt * 512:(jt + 1) * 512],
                            ones16,
                            b_sb[0:1, jt * 512:(jt + 1) * 512],
                            start=False, stop=True)

                # tensor_tensor can read at most one PSUM operand: move gx to SBUF
                gx_sb = big.tile([B, 3 * HID], F32)
                nc.scalar.copy(gx_sb, gx)
                xr, xz, xn = (gx_sb[:, i * 512:(i + 1) * 512] for i in range(3))
                hr, hz, hn = (gh[:, i * 512:(i + 1) * 512] for i in range(3))
                nc.vector.tensor_add(r_sb, xr, hr)
                nc.scalar.activation(r_sb, r_sb, AF.Sigmoid)
                nc.vector.tensor_add(z_sb, xz, hz)
                nc.scalar.activation(z_sb, z_sb, AF.Sigmoid)
                nc.vector.tensor_mul(r_sb, r_sb, hn)       # r <- r * hn
                nc.vector.tensor_add(n_sb, xn, r_sb)
                nc.scalar.activation(n_sb, n_sb, AF.Tanh)
                nc.vector.tensor_sub(h1_sb, h1_sb, n_sb)   # h1 <- h1 - n
                nc.vector.tensor_mul(h1_sb, z_sb, h1_sb)   # h1 <- z * (h1 - n)
                nc.vector.tensor_add(h2_sb, n_sb, h1_sb)   # h2 = n + z*(h1-n)
                nc.sync.dma_start(h2o, h2_sb)

                for c in range(NC4):
                    tp = pg.tile([128, B], F32, tag="tp", bufs=2, name=f"tp2{c}")
                    nc.tensor.transpose(tp, h2_sb[0:B, c * 128:(c + 1) * 128],
                                        ident[0:B, 0:B])
                    nc.scalar.activation(h2T[:, c, :], tp, AF.Copy,
                                         scale=AFF_SCALE)

                s_ps = pg.tile([B, CIN], F32, tag="tp", bufs=2, name="s_ps")
                for c in range(NC4):
                    af_c = gw.tile([128, CIN], F32, tag="gw", name=f"afc{c}")
                    nc.sync.dma_start(af_c, afw[c * 128:(c + 1) * 128, :])
                    nc.tensor.matmul(s_ps, h2T[:, c, :],
                                     af_c,
                                     start=(c == 0), stop=False)
                nc.tensor.matmul(s_ps, ones1, afb_sb,
                                 start=False, stop=True)

                # reuse r_sb (exp) and z_sb (style) — both dead after gates
                e_sb, sty_sb = r_sb, z_sb
                nc.vector.tensor_reduce(negmax, s_ps, axis=AX.X, op=ALU.max,
                                        negate=True)
                nc.scalar.activation(e_sb, s_ps, AF.Exp, bias=negmax,
                                     accum_out=sumexp)
                nc.vector.reciprocal(recip, sumexp)
                nc.vector.tensor_scalar_mul(sty_sb, e_sb, recip)
                nc.scalar.activation(sty_sb, sty_sb, AF.Sqrt)
                nc.sync.dma_start(stylo, sty_sb)

                # per-core style column: selc[:, c] = style[my_b, c*128:...]
                sel_ps = pg.tile([128, NC4], F32, tag="tp", bufs=2, name="sel_ps")
                for c in range(NC4):
                    nc.tensor.matmul(sel_ps[:, c:c + 1],
                                     sty_sb[0:B, c * 128:(c + 1) * 128],
                                     bsel_sb, start=True, stop=True)
                nc.scalar.copy(selc, sel_ps)

            # ---- big DMAs: conv weights + padded input, interleaved by
            # chunk so chunk 0 of both lands first (emitted after the GRU
            # weight DMAs, which gate the style critical path) ------------
            wt_all = big.tile([128, NC4, 9, COUT], BF16)
            xpad = big.tile([128, NC4, HP, WP], BF16)
            nc.vector.memset(xpad[:, :, 0, :], 0.0)
            nc.vector.memset(xpad[:, :, HP - 1, :], 0.0)
            nc.vector.memset(xpad[:, :, :, 0], 0.0)
            nc.vector.memset(xpad[:, :, :, WP - 1], 0.0)
            xin4 = xin.rearrange("(c p) h w -> c p h w", p=128)
            for c in range(NC4):
                nc.sync.dma_start(wt_all[:, c, :, :], wt_d[c])
                nc.sync.dma_start(xpad[:, c, 1:1 + H, 1:1 + W], xin4[c])

            # ---- demodulation + style fold into weights ----------------
            acc = big.tile([128, NC4, COUT], F32)
            for c in range(NC4):
                acc_c = acc[:, c, :]
                adder = nc.vector.tensor_add if c < 2 else nc.gpsimd.tensor_add
                nc.scalar.square(acc_c, wt_all[:, c, 0, :])
                for k in range(1, 9):
                    tmp = sqp.tile([128, COUT], F32, tag="sq", name=f"sq{c}_{k}")
                    nc.scalar.square(tmp, wt_all[:, c, k, :])
                    adder(acc_c, acc_c, tmp)
                nc.scalar.activation(acc_c, acc_c, AF.Sqrt, bias=epsb)
                nc.vector.reciprocal(acc_c, acc_c)
                for k in range(9):
                    nc.vector.tensor_mul(wt_all[:, c, k, :], wt_all[:, c, k, :],
                                         acc_c)
            # style folded into the padded input instead of the weights:
            # per-partition scale of each ci chunk (borders stay zero)
            for c in range(NC4):
                nc.scalar.activation(xpad[:, c, 1:1 + H, 1:1 + W],
                                     xpad[:, c, 1:1 + H, 1:1 + W],
                                     AF.Copy, scale=selc[:, c:c + 1])

            # ---- conv: implicit GEMM -----------------------------------
            with tc.tile_pool(name="pc", bufs=8, space="PSUM") as pc:
                for m in range(NC4):            # co chunk
                    for half in range(2):       # 4 psum banks per half
                        ns = [half * 4 + i for i in range(4)]
                        ps = {n: pc.tile([128, 512], F32, tag="conv",
                                         name=f"cps{m}_{n}") for n in ns}
                        for n in ns:
                            nc.tensor.matmul(ps[n], zw[0:1, 0:128],
                                             zw[0:1, 0:512],
                                             start=True, stop=False)
                        step = 0
                        for c in range(NC4):
                            for k in range(9):
                                ky, kx = k // 3, k % 3
                                lhsT = wt_all[:, c, k,
                                              m * 128:(m + 1) * 128]
                                for n in ns:
                                    rhs = xpad[:, c,
                                               ky + n * NROW: ky + n * NROW + NROW,
                                               kx: kx + W]
                                    nc.tensor.matmul(ps[n], lhsT, rhs,
                                                     start=False,
                                                     stop=(step == 35))
                                step += 1
                        for n in ns:
                            stage = stp.tile([128, 512], F32, tag="stage",
                                             name=f"stg{m}_{n}")
                            nc.vector.tensor_copy(stage, ps[n])
                            nc.sync.dma_start(
                                yflat[m * 128:(m + 1) * 128,
                                      n * 512:(n + 1) * 512], stage)

    nc.compile()
    return nc


def _prep(h1, input, current_w_style, conv_weight, gru_w_ih, gru_w_hh,
          gru_b_ih, gru_b_hh, aff_w, aff_b):
    import ml_dtypes
    f = np.float32
    # conv weight -> [ci_chunk, ci_in_chunk(128), tap(9), co] contiguous
    wt = np.ascontiguousarray(
        conv_weight.astype(f).transpose(2, 3, 1, 0)      # (ky, kx, ci, co)
        .reshape(9, NC4, 128, COUT).transpose(1, 2, 0, 3)
        .astype(ml_dtypes.bfloat16))
    common = {
        "wt": wt,
        "wih": np.ascontiguousarray(gru_w_ih.astype(f).T.astype(np.float16)),
        "whh": np.ascontiguousarray(gru_w_hh.astype(f).T.astype(np.float16)),
        "afw": np.ascontiguousarray(aff_w.astype(f).T),
        "bih": np.ascontiguousarray(gru_b_ih.astype(np.float16).reshape(1, -1)),
        "bhh": np.ascontiguousarray(gru_b_hh.astype(np.float16).reshape(1, -1)),
        "afb": np.ascontiguousarray(aff_b.astype(f).reshape(1, -1)),
        "x8": np.ascontiguousarray(current_w_style.astype(f)),
        "h18": np.ascontiguousarray(h1.astype(f)),
    }
    in_maps = []
    for b in range(B):
        sel = np.zeros((B, 1), f)
        sel[b, 0] = 1.0
        m = dict(common)
        m["xin"] = np.ascontiguousarray(
            input[b].astype(f).astype(ml_dtypes.bfloat16))
        m["bsel"] = sel
        in_maps.append(m)
    return in_maps


def kernel(h1, input, current_w_style, conv_weight, gru_w_ih, gru_w_hh,
           gru_b_ih, gru_b_hh, aff_w, aff_b):
    global LAST_RESULTS
    if "nc" not in _CACHE:
        _CACHE["nc"] = _build()
    nc = _CACHE["nc"]
    in_maps = _prep(h1, input, current_w_style, conv_weight, gru_w_ih,
                    gru_w_hh, gru_b_ih, gru_b_hh, aff_w, aff_b)
    res = run_bass_kernel_spmd(nc, in_maps, core_ids=list(range(B)))
    LAST_RESULTS = res
    out = np.stack([res.results[b]["yout"] for b in range(B)], axis=0)
    h2 = res.results[0]["h2o"]
    style = res.results[0]["stylo"]
    return h2, out, style


# revision 17
# speedup vs baseline: 1.3476x; 1.0366x over previous
"""TRN2 Bass/Tile kernel: GRU-modulated 3x3 conv (B=8, C=512, 64x64).

Sharding: data-parallel over batch across 8 NeuronCores (1 sample/core).
Each core redundantly computes the tiny GRU + affine + softmax style path
for the full batch (needs ~15us of PE) and then runs its own sample's
512->512 3x3 conv as an implicit GEMM: 9 taps x 4 ci-chunks accumulated
into PSUM over a zero-padded 66x66 input image held in SBUF.

The per-sample style scale and the (style-independent) demodulation factor
are folded into the conv weights on-device; conv matmuls run in float32r
(single-pass FP22 matmul, full PE rate at N=512).

Self-contained: hardcodes all shapes; host-side numpy does only layout
prep (transposes / slicing / gather).
"""

import numpy as np

import concourse.bacc as bacc
import concourse.mybir as mybir
from concourse import tile, masks
from concourse.bass_utils import run_bass_kernel_spmd

F32 = mybir.dt.float32
F32R = mybir.dt.float32r
BF16 = mybir.dt.bfloat16
F16 = mybir.dt.float16
AF = mybir.ActivationFunctionType
ALU = mybir.AluOpType
AX = mybir.AxisListType

B, CIN, COUT, KK, H, W = 8, 512, 512, 3, 64, 64
HID = 512
EPS = 1e-8
AFF_SCALE = float(1.0 / np.sqrt(HID))
NC4 = 4           # 512 / 128 chunks
HP, WP = H + 2, W + 2   # 66, 66
NROW = 8          # image rows per psum tile (8*64 = 512 = one PSUM bank)

_CACHE = {}
LAST_RESULTS = None


def _build():
    nc = bacc.Bacc("TRN2", target_bir_lowering=False, debug=False, num_devices=8)

    # ---- DRAM I/O ------------------------------------------------------
    xin = nc.dram_tensor("xin", [CIN, H, W], BF16, kind="ExternalInput").ap()
    wt_d = nc.dram_tensor("wt", [NC4, 128, 9, COUT], BF16, kind="ExternalInput").ap()
    wih = nc.dram_tensor("wih", [HID, 3 * HID], F16, kind="ExternalInput").ap()
    whh = nc.dram_tensor("whh", [HID, 3 * HID], F16, kind="ExternalInput").ap()
    afw = nc.dram_tensor("afw", [HID, CIN], F32, kind="ExternalInput").ap()
    bih = nc.dram_tensor("bih", [1, 3 * HID], F16, kind="ExternalInput").ap()
    bhh = nc.dram_tensor("bhh", [1, 3 * HID], F16, kind="ExternalInput").ap()
    afb = nc.dram_tensor("afb", [1, CIN], F32, kind="ExternalInput").ap()
    x8 = nc.dram_tensor("x8", [B, HID], F32, kind="ExternalInput").ap()
    h18 = nc.dram_tensor("h18", [B, HID], F32, kind="ExternalInput").ap()
    bsel = nc.dram_tensor("bsel", [B, 1], F32, kind="ExternalInput").ap()

    yout = nc.dram_tensor("yout", [COUT, H, W], F32, kind="ExternalOutput").ap()
    h2o = nc.dram_tensor("h2o", [B, HID], F32, kind="ExternalOutput").ap()
    stylo = nc.dram_tensor("stylo", [B, CIN], F32, kind="ExternalOutput").ap()

    yflat = yout.rearrange("co h w -> co (h w)")

    with tile.TileContext(nc) as tc:
        with tc.tile_pool(name="big", bufs=1) as big, \
             tc.tile_pool(name="gw", bufs=5) as gw, \
             tc.tile_pool(name="sq", bufs=8) as sqp, \
             tc.tile_pool(name="st", bufs=3) as stp:

            # ---- constants + small input DMAs (critical path first) ----
            ident = big.tile([128, 128], F32)
            masks.make_identity(nc, ident)
            ones1 = big.tile([1, B], F32)
            nc.vector.memset(ones1, 1.0)
            ones16 = big.tile([1, B], F16)
            nc.vector.memset(ones16, 1.0)
            epsb = big.tile([128, 1], F32)
            nc.vector.memset(epsb, EPS)
            # f32r matmuls drop the start=True (first) contribution on HW:
            # clear each PSUM accumulation group with a zero-weight dummy.
            zw = big.tile([1, 512], BF16)
            nc.vector.memset(zw, 0.0)

            x8_sb = big.tile([B, HID], F32)
            nc.sync.dma_start(x8_sb, x8)
            h1_sb = big.tile([B, HID], F32)
            nc.sync.dma_start(h1_sb, h18)
            bih_sb = big.tile([1, 3 * HID], F16)
            nc.sync.dma_start(bih_sb, bih)
            bhh_sb = big.tile([1, 3 * HID], F16)
            nc.sync.dma_start(bhh_sb, bhh)
            afb_sb = big.tile([1, CIN], F32)
            nc.sync.dma_start(afb_sb, afb)
            bsel_sb = big.tile([B, 1], F32)
            nc.sync.dma_start(bsel_sb, bsel)

            # ---- GRU + affine + softmax(style) -------------------------
            xT = big.tile([128, NC4, B], F16)
            h1T = big.tile([128, NC4, B], F16)
            h2T = big.tile([128, NC4, B], F32)
            selc = big.tile([128, NC4], F32)

            r_sb = big.tile([B, HID], F32)
            z_sb = big.tile([B, HID], F32)
            n_sb = big.tile([B, HID], F32)
            h2_sb = big.tile([B, HID], F32)
            negmax = big.tile([B, 1], F32)
            sumexp = big.tile([B, 1], F32)
            recip = big.tile([B, 1], F32)

            with tc.tile_pool(name="pg", bufs=1, space="PSUM") as pg:
                for c in range(NC4):
                    tp = pg.tile([128, B], F32, tag="tp", bufs=2, name=f"tpx{c}")
                    nc.tensor.transpose(tp, x8_sb[0:B, c * 128:(c + 1) * 128],
                                        ident[0:B, 0:B])
                    nc.scalar.copy(xT[:, c, :], tp)
                for c in range(NC4):
                    tp = pg.tile([128, B], F32, tag="tp", bufs=2, name=f"tph{c}")
                    nc.tensor.transpose(tp, h1_sb[0:B, c * 128:(c + 1) * 128],
                                        ident[0:B, 0:B])
                    nc.scalar.copy(h1T[:, c, :], tp)

                gx = pg.tile([B, 3 * HID], F32, tag="g", bufs=2, name="gx")
                gh = pg.tile([B, 3 * HID], F32, tag="g", bufs=2, name="gh")

                for (g_ps, w_dram, lhsT, b_sb) in (
                        (gx, wih, xT, bih_sb), (gh, whh, h1T, bhh_sb)):
                    for c in range(NC4):
                        w_c = gw.tile([128, 3 * HID], F16, tag="gw", name=f"gwc{c}")
                        nc.sync.dma_start(w_c, w_dram[c * 128:(c + 1) * 128, :])
                        for jt in range(3):
                            nc.tensor.matmul(
                                g_ps[:, jt * 512:(jt + 1) * 512],
                                lhsT[:, c, :],
                                w_c[:, jt * 512:(jt + 1) * 512],
                                start=(c == 0), stop=False)
                    for jt in range(3):
                        nc.tensor.matmul(
                            g_ps[:, jt * 512:(jt + 1) * 512],
                            ones16,
                            b_sb[0:1, jt * 512:(jt + 1) * 512],
                            start=False, stop=True)

                # tensor_tensor can read at most one PSUM operand: move gx to SBUF
                gx_sb = big.tile([B, 3 * HID], F32)
                nc.scalar.copy(gx_sb, gx)
                xr, xz, xn = (gx_sb[:, i * 512:(i + 1) * 512] for i in range(3))
                hr, hz, hn = (gh[:, i * 512:(i + 1) * 512] for i in range(3))
                nc.vector.tensor_add(r_sb, xr, hr)
                nc.scalar.activation(r_sb, r_sb, AF.Sigmoid)
                nc.vector.tensor_add(z_sb, xz, hz)
                nc.scalar.activation(z_sb, z_sb, AF.Sigmoid)
                nc.vector.tensor_mul(r_sb, r_sb, hn)       # r <- r * hn
                nc.vector.tensor_add(n_sb, xn, r_sb)
                nc.scalar.activation(n_sb, n_sb, AF.Tanh)
                nc.vector.tensor_sub(h1_sb, h1_sb, n_sb)   # h1 <- h1 - n
                nc.vector.tensor_mul(h1_sb, z_sb, h1_sb)   # h1 <- z * (h1 - n)
                nc.vector.tensor_add(h2_sb, n_sb, h1_sb)   # h2 = n + z*(h1-n)
                nc.sync.dma_start(h2o, h2_sb)

                for c in range(NC4):
                    tp = pg.tile([128, B], F32, tag="tp", bufs=2, name=f"tp2{c}")
                    nc.tensor.transpose(tp, h2_sb[0:B, c * 128:(c + 1) * 128],
                                        ident[0:B, 0:B])
                    nc.scalar.activation(h2T[:, c, :], tp, AF.Copy,
                                         scale=AFF_SCALE)

                s_ps = pg.tile([B, CIN], F32, tag="tp", bufs=2, name="s_ps")
                for c in range(NC4):
                    af_c = gw.tile([128, CIN], F32, tag="gw", name=f"afc{c}")
                    nc.sync.dma_start(af_c, afw[c * 128:(c + 1) * 128, :])
                    nc.tensor.matmul(s_ps, h2T[:, c, :],
                                     af_c,
                                     start=(c == 0), stop=False)
                nc.tensor.matmul(s_ps, ones1, afb_sb,
                                 start=False, stop=True)

                # reuse r_sb (exp) and z_sb (style) — both dead after gates
                e_sb, sty_sb = r_sb, z_sb
                nc.vector.tensor_reduce(negmax, s_ps, axis=AX.X, op=ALU.max,
                                        negate=True)
                nc.scalar.activation(e_sb, s_ps, AF.Exp, bias=negmax,
                                     accum_out=sumexp)
                nc.vector.reciprocal(recip, sumexp)
                nc.vector.tensor_scalar_mul(sty_sb, e_sb, recip)
                nc.scalar.activation(sty_sb, sty_sb, AF.Sqrt)
                nc.sync.dma_start(stylo, sty_sb)

                # per-core style column: selc[:, c] = style[my_b, c*128:...]
                sel_ps = pg.tile([128, NC4], F32, tag="tp", bufs=2, name="sel_ps")
                for c in range(NC4):
                    nc.tensor.matmul(sel_ps[:, c:c + 1],
                                     sty_sb[0:B, c * 128:(c + 1) * 128],
                                     bsel_sb, start=True, stop=True)
                nc.scalar.copy(selc, sel_ps)

            # ---- big DMAs: conv weights + padded input, interleaved by
            # chunk so chunk 0 of both lands first (emitted after the GRU
            # weight DMAs, which gate the style critical path) ------------
            wt_all = big.tile([128, NC4, 9, COUT], BF16)
            xpad = big.tile([128, NC4, HP, WP], BF16)
            nc.vector.memset(xpad[:, :, 0, :], 0.0)
            nc.vector.memset(xpad[:, :, HP - 1, :], 0.0)
            nc.vector.memset(xpad[:, :, :, 0], 0.0)
            nc.vector.memset(xpad[:, :, :, WP - 1], 0.0)
            xin4 = xin.rearrange("(c p) h w -> c p h w", p=128)
            for c in range(NC4):
                nc.sync.dma_start(wt_all[:, c, :, :], wt_d[c])
                nc.sync.dma_start(xpad[:, c, 1:1 + H, 1:1 + W], xin4[c])

            # ---- demodulation + style fold into weights ----------------
            acc = big.tile([128, NC4, COUT], F32)
            for c in range(NC4):
                acc_c = acc[:, c, :]
                adder = nc.vector.tensor_add if c < 2 else nc.gpsimd.tensor_add
                nc.scalar.square(acc_c, wt_all[:, c, 0, :])
                for k in range(1, 9):
                    tmp = sqp.tile([128, COUT], F32, tag="sq", name=f"sq{c}_{k}")
                    nc.scalar.square(tmp, wt_all[:, c, k, :])
                    adder(acc_c, acc_c, tmp)
                nc.scalar.activation(acc_c, acc_c, AF.Sqrt, bias=epsb)
                nc.vector.reciprocal(acc_c, acc_c)
                for k in range(9):
                    nc.vector.tensor_mul(wt_all[:, c, k, :], wt_all[:, c, k, :],
                                         acc_c)
            # style folded into the padded input instead of the weights:
            # per-partition scale of each ci chunk (borders stay zero)
            for c in range(NC4):
                nc.scalar.activation(xpad[:, c, 1:1 + H, 1:1 + W],
                                     xpad[:, c, 1:1 + H, 1:1 + W],
                                     AF.Copy, scale=selc[:, c:c + 1])

            # ---- conv: implicit GEMM -----------------------------------
            with tc.tile_pool(name="pc", bufs=8, space="PSUM") as pc:
                for m in range(NC4):            # co chunk
                    for half in range(2):       # 4 psum banks per half
                        ns = [half * 4 + i for i in range(4)]
                        ps = {n: pc.tile([128, 512], F32, tag="conv",
                                         name=f"cps{m}_{n}") for n in ns}
                        for n in ns:
                            nc.tensor.matmul(ps[n], zw[0:1, 0:128],
                                             zw[0:1, 0:512],
                                             start=True, stop=False)
                        step = 0
                        for c in range(NC4):
                            for k in range(9):
                                ky, kx = k // 3, k % 3
                                lhsT = wt_all[:, c, k,
                                              m * 128:(m + 1) * 128]
                                for n in ns:
                                    rhs = xpad[:, c,
                                               ky + n * NROW: ky + n * NROW + NROW,
                                               kx: kx + W]
                                    nc.tensor.matmul(ps[n], lhsT, rhs,
                                                     start=False,
                                                     stop=(step == 35))
                                step += 1
                        for n in ns:
                            stage = stp.tile([128, 512], F32, tag="stage",
                                             name=f"stg{m}_{n}")
                            nc.vector.tensor_copy(stage, ps[n])
                            nc.sync.dma_start(
                                yflat[m * 128:(m + 1) * 128,
                                      n * 512:(n + 1) * 512], stage)

    nc.compile()
    return nc


def _prep(h1, input, current_w_style, conv_weight, gru_w_ih, gru_w_hh,
          gru_b_ih, gru_b_hh, aff_w, aff_b):
    import ml_dtypes
    f = np.float32
    # conv weight -> [ci_chunk, ci_in_chunk(128), tap(9), co] contiguous
    wt = np.ascontiguousarray(
        conv_weight.astype(f).transpose(2, 3, 1, 0)      # (ky, kx, ci, co)
        .reshape(9, NC4, 128, COUT).transpose(1, 2, 0, 3)
        .astype(ml_dtypes.bfloat16))
    common = {
        "wt": wt,
        "wih": np.ascontiguousarray(gru_w_ih.astype(f).T.astype(np.float16)),
        "whh": np.ascontiguousarray(gru_w_hh.astype(f).T.astype(np.float16)),
        "afw": np.ascontiguousarray(aff_w.astype(f).T),
        "bih": np.ascontiguousarray(gru_b_ih.astype(np.float16).reshape(1, -1)),
        "bhh": np.ascontiguousarray(gru_b_hh.astype(np.float16).reshape(1, -1)),
        "afb": np.ascontiguousarray(aff_b.astype(f).reshape(1, -1)),
        "x8": np.ascontiguousarray(current_w_style.astype(f)),
        "h18": np.ascontiguousarray(h1.astype(f)),
    }
    in_maps = []
    for b in range(B):
        sel = np.zeros((B, 1), f)
        sel[b, 0] = 1.0
        m = dict(common)
        m["xin"] = np.ascontiguousarray(
            input[b].astype(f).astype(ml_dtypes.bfloat16))
        m["bsel"] = sel
        in_maps.append(m)
    return in_maps


def kernel(h1, input, current_w_style, conv_weight, gru_w_ih, gru_w_hh,
           gru_b_ih, gru_b_hh, aff_w, aff_b):
    global LAST_RESULTS
    if "nc" not in _CACHE:
        _CACHE["nc"] = _build()
    nc = _CACHE["nc"]
    in_maps = _prep(h1, input, current_w_style, conv_weight, gru_w_ih,
                    gru_w_hh, gru_b_ih, gru_b_hh, aff_w, aff_b)
    res = run_bass_kernel_spmd(nc, in_maps, core_ids=list(range(B)))
    LAST_RESULTS = res
    out = np.stack([res.results[b]["yout"] for b in range(B)], axis=0)
    h2 = res.results[0]["h2o"]
    style = res.results[0]["stylo"]
    return h2, out, style
